# revision 1
# baseline (speedup 1.0000x reference)
"""CricketHeteroGNN kernel for 8 trn2 NeuronCores.

Strategy (memory-regime): all gather/scatter index structure is resolved on
the host into sorted-run segment reductions; the dense per-node compute is
algebraically reassociated so every segment aggregation happens on RAW node
features (seg_mean(x[src] @ W) == seg_mean(x[src]) @ W), which shrinks the
matmul work to node-count rather than edge-count rows.

This file is self-contained (numpy only) so the grading harness can call
kernel(**inputs) in a fresh directory.
"""
import numpy as np

H = 128
N_BALL, N_QUERY, N_PLAYER, N_VENUE, N_TEAM = 200000, 8192, 180224, 8192, 16384
L = 3


def _layer_norm(x, g, b, eps=1e-5):
    m = x.mean(-1, keepdims=True)
    v = ((x - m) ** 2).mean(-1, keepdims=True)
    return (x - m) / np.sqrt(v + eps) * g + b


def _gelu(x):
    # jax.nn.gelu default (approximate=True, tanh form)
    c = np.sqrt(2.0 / np.pi).astype(np.float32)
    return 0.5 * x * (1.0 + np.tanh(c * (x + 0.044715 * x * x * x)))


def _seg_sum_count(msgs, dst, n):
    """Sorted-run segment sum via reduceat (fast, vectorized)."""
    order = np.argsort(dst, kind="stable")
    sd = dst[order]
    sm = msgs[order]
    # run boundaries
    starts = np.flatnonzero(np.r_[True, sd[1:] != sd[:-1]])
    uniq = sd[starts]
    sums = np.add.reduceat(sm, starts, axis=0)
    out = np.zeros((n, msgs.shape[1]), dtype=np.float32)
    out[uniq] = sums
    cnt = np.bincount(dst, minlength=n).astype(np.float32)
    return out, cnt


def _seg_mean(msgs, dst, n):
    s, c = _seg_sum_count(msgs, dst, n)
    return s / np.maximum(c, 1.0)[:, None]


def kernel(**inputs):
    ins = {k: np.asarray(v) for k, v in inputs.items()}
    f32 = np.float32

    ball_x = ins["ball_x"].astype(f32)
    query_x = ins["query_x"].astype(f32)
    player_table = ins["player_table"].astype(f32)
    role_table = ins["role_table"].astype(f32)
    venue_table = ins["venue_table"].astype(f32)
    team_table = ins["team_table"].astype(f32)

    venue_id = ins["venue_id"].astype(np.int64)
    team_id = ins["team_id"].astype(np.int64)
    player_id = ins["player_id"].astype(np.int64)
    role_id = ins["role_id"].astype(np.int64)
    bb_src, bb_dst = ins["bb_src"].astype(np.int64), ins["bb_dst"].astype(np.int64)
    pb_src, pb_dst = ins["pb_src"].astype(np.int64), ins["pb_dst"].astype(np.int64)
    bq_src, bq_dst = ins["bq_src"].astype(np.int64), ins["bq_dst"].astype(np.int64)
    vq_src, vq_dst = ins["vq_src"].astype(np.int64), ins["vq_dst"].astype(np.int64)
    tq_src, tq_dst = ins["tq_src"].astype(np.int64), ins["tq_dst"].astype(np.int64)

    n_ball, n_query = ball_x.shape[0], query_x.shape[0]

    # --- node encoders ---
    x_venue = venue_table[venue_id] @ ins["enc_W_venue"].astype(f32) + ins["enc_b_venue"].astype(f32)
    x_team = team_table[team_id] @ ins["enc_W_team"].astype(f32) + ins["enc_b_team"].astype(f32)
    x_player = (
        np.concatenate([player_table[player_id], role_table[role_id]], -1)
        @ ins["enc_W_player"].astype(f32)
        + ins["enc_b_player"].astype(f32)
    )
    x_ball = ball_x @ ins["enc_W_ball"].astype(f32) + ins["enc_b_ball"].astype(f32)
    x_query = query_x @ ins["enc_W_query"].astype(f32) + ins["enc_b_query"].astype(f32)

    conv_rel_W = ins["conv_rel_W"].astype(f32)
    conv_self_W = ins["conv_self_W"].astype(f32)
    conv_self_b = ins["conv_self_b"].astype(f32)
    ln_g, ln_b = ins["ln_g"].astype(f32), ins["ln_b"].astype(f32)

    # Layer-invariant aggregations: players / venues / teams never update,
    # so seg_mean(x[src] @ W_l) = seg_mean(x[src]) @ W_l and the aggregation
    # is computed once (the per-layer matmul stays tiny: [N,128]@[128,128]).
    agg_pb = _seg_mean(x_player[pb_src], pb_dst, n_ball)
    agg_vq = _seg_mean(x_venue[vq_src], vq_dst, n_query)
    agg_tq = _seg_mean(x_team[tq_src], tq_dst, n_query)

    for l in range(conv_rel_W.shape[0]):
        a_bb = _seg_mean(x_ball[bb_src], bb_dst, n_ball)
        a_bq = _seg_mean(x_ball[bq_src], bq_dst, n_query)
        m_bb = a_bb @ conv_rel_W[l, 0]
        m_pb = agg_pb @ conv_rel_W[l, 1]
        m_bq = a_bq @ conv_rel_W[l, 2]
        m_vq = agg_vq @ conv_rel_W[l, 3]
        m_tq = agg_tq @ conv_rel_W[l, 4]
        ball_new = _layer_norm(
            np.maximum(x_ball @ conv_self_W[l, 0] + conv_self_b[l, 0] + m_bb + m_pb, 0.0),
            ln_g[l, 0], ln_b[l, 0],
        )
        query_new = _layer_norm(
            np.maximum(x_query @ conv_self_W[l, 1] + conv_self_b[l, 1] + m_bq + m_vq + m_tq, 0.0),
            ln_g[l, 1], ln_b[l, 1],
        )
        x_ball, x_query = ball_new, query_new

    # --- predictor ---
    h = _gelu(_layer_norm(x_query @ ins["pred_W1"].astype(f32) + ins["pred_b1"].astype(f32),
                          ins["pred_g1"].astype(f32), ins["pred_be1"].astype(f32)))
    h = _gelu(_layer_norm(h @ ins["pred_W2"].astype(f32) + ins["pred_b2"].astype(f32),
                          ins["pred_g2"].astype(f32), ins["pred_be2"].astype(f32)))
    logits = h @ ins["pred_W3"].astype(f32) + ins["pred_b3"].astype(f32)
    return logits.astype(np.float32)



# revision 2
# speedup vs baseline: 6066.9240x; 6066.9240x over previous
"""CricketHeteroGNN kernel: 8-core trn2 Bass implementation (see gnn_impl).

kernel(**inputs) -> [8192, 7] f32 logits.
"""
import sys

sys.path.insert(0, "/root/problem")

import numpy as np

_CACHE = {}
LAST_EXEC_TIME_NS = None


def kernel(**inputs):
    global LAST_EXEC_TIME_NS
    import gnn_impl

    prep = gnn_impl.HostPrep(inputs, gnn_impl.FULL_CFG)
    nc = _CACHE.get("nc")
    if nc is None:
        nc = gnn_impl.build_program(prep)
        _CACHE["nc"] = nc
    import os
    trace = os.environ.get("GNN_TRACE", "0") == "1"
    logits, res = gnn_impl.run(prep, nc, trace=trace)
    if res.exec_time_ns:
        LAST_EXEC_TIME_NS = res.exec_time_ns
    return logits
